# revision 14
# baseline (speedup 1.0000x reference)
"""AttentionPool Trainium2 kernel (v2 — transposed fp16 streaming design).

Computes, for x (B,T,m), W1 (m,m), W2 (m,m), vm (1,m):
    h      = tanh(x @ W1 + vm @ W2)          (B,T,m)
    scores = h @ vm[0]                       (B,T,1)
    w      = softmax(scores, axis=T)
    out    = sum(x * w, axis=T, keepdims)    (B,1,m)

Sharding: data-parallel over B across 8 NeuronCores (2 examples/core);
weights replicated.  Host pre-stages x as fp16 transposed to [B, m, T]
so the device reads half the bytes and needs no on-device transpose or
cast (both were dominant costs in v1: GPSIMD cast 115us, PE transposes
+ DVE psum copies, DVE f32 pooling 137us).

Per-core dataflow, tile = 1024 t-columns (8 tiles/example):
  DMA x^T tile [128m x 2mh x 1024t] fp16 (16KB contiguous rows)
  PE:  h^T[nh] = sum_mh W1[mh,nh]^T x^T[mh]      (2 psum tiles, fp16 1cyc/row)
  ACT: hs[nh]  = tanh(h^T + c[nh])               (per-partition bias, fp16 out)
  PE:  s_rep   = sum_nh vrep[nh]^T hs[nh]        (vm replicated to 128 identical
       stationary columns -> psum holds s broadcast across all 128 partitions)
  ACT: e = exp(s_rep - 4), accum_out Z-partial   (fp16 e, bias keeps e in fp16
       range; softmax shift cancels after normalization)
  DVE: per mh: tensor_tensor_reduce(x^T tile * e) -> acc partial [128,1] f32
       (all-fp16 operands -> DVE high-perf mode; accumulate over free dim)
  finalize: Z = sum(Z partials), acc = sum(partials), out = acc / Z, DMA out.

Softmax needs no max-subtraction: |scores| <= ||vm||_1 ~ 13, and with the
-4 bias exp(s-4) <= e^9 stays in fp16 range with margin.
"""

import numpy as np

import concourse.bass as bass
import concourse.tile as tile
from concourse import bacc, mybir
from concourse.bass_utils import run_bass_kernel_spmd

FP32 = mybir.dt.float32
FP16 = mybir.dt.float16
AF = mybir.ActivationFunctionType
ALU = mybir.AluOpType

N_CORES = 8
B = 16
B_PER_CORE = B // N_CORES  # 2
T = 8192
M = 256
P = 128
PB = 512             # psum bank width in f32 (matmul output limit)
CT = 1024            # t-columns per compute tile (ACT ops span 2 banks)
NTILE = T // CT      # compute tiles per example
GRP = 2048           # t-columns per pooling group (amortizes DVE overhead)
NGRP = T // GRP      # pooling groups per example
DMA_CHUNK = 2048     # t-columns per input DMA
S_BIAS = -4.0        # exp(s + S_BIAS): keeps e in fp16 range; cancels in w


def _build_program() -> bass.Bass:
    nc = bacc.Bacc("TRN2", target_bir_lowering=False, debug=False)

    xt = nc.dram_tensor("xt", [B_PER_CORE, M, T], FP16, kind="ExternalInput")
    W1 = nc.dram_tensor("W1", [M, M], FP32, kind="ExternalInput")
    W2 = nc.dram_tensor("W2", [M, M], FP32, kind="ExternalInput")
    vm = nc.dram_tensor("vm", [1, M], FP32, kind="ExternalInput")
    out = nc.dram_tensor("out", [B_PER_CORE, M], FP32, kind="ExternalOutput")

    with tile.TileContext(nc) as tc:
        with (
            tc.tile_pool(name="setup", bufs=1) as setup,
            tc.tile_pool(name="xin", bufs=B_PER_CORE) as x_pool,
            tc.tile_pool(name="hps", bufs=3, space="PSUM") as h_psum,
            tc.tile_pool(name="sps", bufs=1, space="PSUM") as s_psum,
            tc.tile_pool(name="hsb", bufs=4) as hs_pool,
            tc.tile_pool(name="eee", bufs=2) as e_pool,
            tc.tile_pool(name="scr", bufs=1) as scr_pool,
            tc.tile_pool(name="acc", bufs=2) as acc_pool,
            tc.tile_pool(name="fin", bufs=2) as fin_pool,
        ):
            # ---------------- input DMA (issued first: x is the long pole) --
            xt_sb = []
            for b in range(B_PER_CORE):
                xtile = x_pool.tile([P, 2, T], FP16)
                src = xt[b].rearrange("(a p) t -> p a t", p=P)
                for q in range(T // DMA_CHUNK):
                    sl = slice(q * DMA_CHUNK, (q + 1) * DMA_CHUNK)
                    nc.sync.dma_start(out=xtile[:, :, sl], in_=src[:, :, sl])
                xt_sb.append(xtile)

            # ---------------- setup ----------------
            # W1 blocks: w1b[p, mh, n] = W1[mh*128+p, n], cast fp16
            w1f = setup.tile([P, 2, M], FP32)
            nc.sync.dma_start(out=w1f, in_=W1.rearrange("(a p) n -> p a n", p=P))
            w1b = setup.tile([P, 2, M], FP16)
            nc.vector.tensor_copy(w1b, w1f)

            # W2 blocks (f32, setup only)
            w2f = setup.tile([P, 2, M], FP32)
            nc.sync.dma_start(out=w2f, in_=W2.rearrange("(a p) n -> p a n", p=P))

            # vm transposed: vmt_f[p, nh] = vm[0, nh*128+p]
            vmt_f = setup.tile([P, 2], FP32)
            nc.sync.dma_start(out=vmt_f, in_=vm[0].rearrange("(a p) -> p a", p=P))

            # c = vm @ W2, computed transposed: c_sb[p, nh] = c[nh*128+p]
            c_ps = s_psum.tile([P, 2], FP32, tag="sps")
            for nh in range(2):
                for mh in range(2):
                    nc.tensor.matmul(
                        c_ps[:, nh : nh + 1],
                        lhsT=w2f[:, mh, nh * P : (nh + 1) * P],
                        rhs=vmt_f[:, mh : mh + 1],
                        start=(mh == 0),
                        stop=(mh == 1),
                    )
            c_sb = setup.tile([P, 2], FP32)
            nc.vector.tensor_copy(c_sb, c_ps)

            # vrep[p, nh, j] = vm[nh*128+p] for all j: replicated stationary
            # so the score matmul broadcasts s across all 128 psum partitions.
            ones_h = setup.tile([P, P], FP16)
            nc.vector.memset(ones_h, 1.0)
            sbias = setup.tile([P, 1], FP32)
            nc.vector.memset(sbias, S_BIAS)
            vrep = setup.tile([P, 2, P], FP16)
            for nh in range(2):
                nc.vector.tensor_scalar_mul(
                    vrep[:, nh, :], ones_h, vmt_f[:, nh : nh + 1]
                )

            # ---------------- main loop ----------------
            outsb = fin_pool.tile([P, B_PER_CORE, 2], FP32)
            for b in range(B_PER_CORE):
                acc0 = acc_pool.tile([P, NGRP + 1], FP32)  # mh=0 pool partials
                acc1 = acc_pool.tile([P, NGRP + 1], FP32)  # mh=1 pool partials
                z_t = acc_pool.tile([P, NTILE], FP32)      # Z partials

                for g in range(NGRP):
                    # e for the whole 2048-col group (pooling reads it wide)
                    e16 = e_pool.tile([P, GRP], FP16)

                    for jj in range(GRP // CT):
                        j = g * (GRP // CT) + jj
                        NQ = CT // PB  # 512-col psum banks per compute tile

                        # h^T per n-half: accumulate over m-halves
                        # (psum tile spans NQ banks; matmuls write 512-col
                        # slices, tanh reads the whole tile in one op)
                        hs = []
                        for nh in range(2):
                            hp = h_psum.tile([P, NQ, PB], FP32)
                            for q in range(NQ):
                                qcols = slice(
                                    (j * NQ + q) * PB, (j * NQ + q + 1) * PB
                                )
                                for mh in range(2):
                                    nc.tensor.matmul(
                                        hp[:, q, :],
                                        lhsT=w1b[:, mh, nh * P : (nh + 1) * P],
                                        rhs=xt_sb[b][:, mh, qcols],
                                        start=(mh == 0),
                                        stop=(mh == 1),
                                    )
                            h16 = hs_pool.tile([P, NQ, PB], FP16)
                            nc.scalar.activation(
                                h16, hp, AF.Tanh, bias=c_sb[:, nh : nh + 1]
                            )
                            hs.append(h16)

                        # scores broadcast to all partitions via replicated vm
                        sp = s_psum.tile([P, NQ, PB], FP32, tag="sps")
                        for q in range(NQ):
                            for nh in range(2):
                                nc.tensor.matmul(
                                    sp[:, q, :],
                                    lhsT=vrep[:, nh, :],
                                    rhs=hs[nh][:, q, :],
                                    start=(nh == 0),
                                    stop=(nh == 1),
                                )

                        # e = exp(s - 4) in fp16 (+ per-tile Z partial)
                        nc.scalar.activation(
                            e16[:, jj * CT : (jj + 1) * CT], sp,
                            AF.Exp, bias=sbias, accum_out=z_t[:, j : j + 1],
                        )

                    # pooling: acc[m] partial = sum_t x^T[m, t] * e[t].
                    # Last group split in half so the tail s_t_t starts
                    # right after the second-to-last exp.
                    if g < NGRP - 1:
                        spans = [(slice(g * GRP, (g + 1) * GRP),
                                  slice(0, GRP), g)]
                    else:
                        half = GRP // 2
                        spans = [
                            (slice(g * GRP, g * GRP + half),
                             slice(0, half), g),
                            (slice(g * GRP + half, (g + 1) * GRP),
                             slice(half, GRP), g + 1),
                        ]
                    for xcols, ecols, col in spans:
                        for mh, acc in ((0, acc0), (1, acc1)):
                            scr = scr_pool.tile([P, GRP], FP16, tag="scr")
                            nc.vector.scalar_tensor_tensor(
                                out=scr[:, ecols],
                                in0=xt_sb[b][:, mh, xcols],
                                scalar=0.0,
                                in1=e16[:, ecols],
                                op0=ALU.bypass,
                                op1=ALU.mult,
                                accum_out=acc[:, col : col + 1],
                            )

                # ---- finalize example ----
                zs = fin_pool.tile([P, 1], FP32)
                nc.vector.reduce_sum(zs, z_t, axis=mybir.AxisListType.X)
                rz = fin_pool.tile([P, 1], FP32)
                nc.vector.reciprocal(rz, zs)
                asum = fin_pool.tile([P, 2], FP32)
                nc.vector.reduce_sum(
                    asum[:, 0:1], acc0, axis=mybir.AxisListType.X
                )
                nc.vector.reduce_sum(
                    asum[:, 1:2], acc1, axis=mybir.AxisListType.X
                )
                nc.vector.tensor_scalar_mul(outsb[:, b, :], asum, rz)

            nc.sync.dma_start(
                out=out.rearrange("b (a p) -> p b a", p=P), in_=outsb
            )

    return nc


_PROGRAM_CACHE: list = []


def _get_program() -> bass.Bass:
    if not _PROGRAM_CACHE:
        nc = _build_program()
        nc.finalize()
        _PROGRAM_CACHE.append(nc)
    return _PROGRAM_CACHE[0]


def _make_in_maps(inputs):
    x = np.asarray(inputs["x"])
    W1 = np.ascontiguousarray(inputs["W1"], dtype=np.float32)
    W2 = np.ascontiguousarray(inputs["W2"], dtype=np.float32)
    vm = np.ascontiguousarray(inputs["vm"], dtype=np.float32)
    # Host staging: fp16 cast + transpose to [B, m, T] (the kernel's chosen
    # input layout — halves HBM traffic and removes on-device transposes).
    xt = np.ascontiguousarray(x.astype(np.float16).transpose(0, 2, 1))
    return [
        {
            "xt": xt[i * B_PER_CORE : (i + 1) * B_PER_CORE],
            "W1": W1,
            "W2": W2,
            "vm": vm,
        }
        for i in range(N_CORES)
    ]


def kernel(x, W1, W2, vm):
    nc = _get_program()
    core_ids = list(range(N_CORES))
    in_maps = _make_in_maps({"x": x, "W1": W1, "W2": W2, "vm": vm})
    res = run_bass_kernel_spmd(nc, in_maps, core_ids)
    out = np.concatenate([res.results[i]["out"] for i in range(N_CORES)], axis=0)
    return out.reshape(B, 1, M)
